# revision 40
# baseline (speedup 1.0000x reference)
"""Bass/Trainium2 kernel for nn_Encoder_32452772888844.

64 independent 2-layer LSTM(256) encoders + per-group Linear(256,256),
then shared heads:
  lin1  = fc @ W1.T + b1
  delta = softmax(lin1, axis=0)   (over the 64 groups)
  beta  = softplus(fc @ W2.T + b2)
  gamma = lin1 @ Wd.T + bd
Sharding: pure group parallelism — 8 groups per NeuronCore; each core
computes everything for its groups including exp(lin1); the softmax
normalization (a sum over the 64-group axis) is applied on the host.

Device-side formulation (per core):
  - All matmuls are weight-stationary: lhsT tiles [K=128, M=128] are
    (transposed) weight blocks, the moving operand is the activation
    vector/sequence. Gate results land in PSUM with the hidden dim on
    partitions, which makes the LSTM cell elementwise work efficient.
  - Weights are cast to bf16 on the host (PSUM accumulates in fp32).
  - x-projections (with biases) for all 10 timesteps are batched up
    front per layer; at each recurrence step an identity matmul
    preloads x-projection+bias into the gates PSUM region and the
    h-recurrence matmuls accumulate on top, so the ScalarE activations
    read gates straight out of PSUM (no elementwise add on the
    critical path).
  - Groups are processed in two halves per step so one half's cell
    math overlaps the other half's matmuls.
  - Gate chunk order is host-permuted from PyTorch's (i,f,g,o) to
    (i,i,f,f,o,o,g,g) 128-blocks so sigmoid covers one contiguous
    slice.
"""

import numpy as np
import ml_dtypes

T = 10
IN = 256
H = 256
G = 64
NCORES = 8
GPC = G // NCORES  # groups per core
A = 2   # 128-halves of 256
C8 = 8  # 128-chunks of 1024
C2 = 2  # 128-chunks of 256

BF16 = ml_dtypes.bfloat16
SBF_N = GPC * A * T + 128 + 2 * (A * C2 * 128) + A + 2 * GPC * C8 * T + GPC * C2 * T
SF32_N = 2 * GPC * C8 + GPC * C2 + C2 + C2 + 1

_COMPILED = {}


def _build_nc():
    import concourse.tile as tile
    from concourse import bacc, mybir

    f32 = mybir.dt.float32
    bf16 = mybir.dt.bfloat16
    Sig = mybir.ActivationFunctionType.Sigmoid
    Tanh = mybir.ActivationFunctionType.Tanh
    Exp = mybir.ActivationFunctionType.Exp
    Ln = mybir.ActivationFunctionType.Ln

    nc = bacc.Bacc(None, target_bir_lowering=False)

    # ---- DRAM parameters (per-core shards, host-prepared layouts) ----
    d_sbf = nc.dram_tensor("sbf", [128, SBF_N], bf16, kind="ExternalInput")
    d_sf32 = nc.dram_tensor("sf32", [128, SF32_N], f32, kind="ExternalInput")
    # wih and whh merged: dim1 -> 0=ih, 1=hh
    d_w = [
        nc.dram_tensor("w0", [128, GPC, 2, A, C8, 128], bf16, kind="ExternalInput"),
        nc.dram_tensor("w1", [128, GPC, 2, A, C8, 128], bf16, kind="ExternalInput"),
    ]
    d_wlin = nc.dram_tensor("wlin", [128, GPC, A, C2, 128], bf16, kind="ExternalInput")

    d_lin1 = nc.dram_tensor("lin1o", [128, C2, GPC, T], f32, kind="ExternalOutput")
    d_expl = nc.dram_tensor("explo", [128, C2, GPC, T], f32, kind="ExternalOutput")
    d_beta = nc.dram_tensor("betao", [128, C2, GPC, T], f32, kind="ExternalOutput")
    d_gamma = nc.dram_tensor("gammao", [1, GPC, T], f32, kind="ExternalOutput")
    d_hn = nc.dram_tensor("hno", [128, 2, GPC, A], f32, kind="ExternalOutput")
    d_cn = nc.dram_tensor("cno", [128, 2, GPC, A], f32, kind="ExternalOutput")

    with tile.TileContext(nc) as tc:
        with (
            tc.tile_pool(name="wpool", bufs=1) as wpool,
            tc.tile_pool(name="apool", bufs=1) as apool,
            tc.tile_pool(name="tpool", bufs=1) as tpool,
            tc.tile_pool(name="pspool", bufs=1, space="PSUM") as pspool,
        ):
            # ---- loads, in compute-consumption order ----
            # All small tensors are host-packed into two tensors (one per
            # dtype) so only two DMA-issue slots sit ahead of the weights.
            s_sbf = apool.tile([128, SBF_N], bf16, tag="sbf", name="s_sbf")
            nc.sync.dma_start(s_sbf[:], d_sbf[:])
            s_sf32 = apool.tile([128, SF32_N], f32, tag="sf32", name="s_sf32")
            nc.sync.dma_start(s_sf32[:], d_sf32[:])
            o = 0
            s_xT = s_sbf[:, o : o + GPC * A * T].rearrange(
                "p (g a t) -> p g a t", g=GPC, a=A); o += GPC * A * T
            s_ident = s_sbf[:, o : o + 128]; o += 128
            s_w1t = s_sbf[:, o : o + A * C2 * 128].rearrange(
                "p (a c m) -> p a c m", a=A, c=C2); o += A * C2 * 128
            s_w2t = s_sbf[:, o : o + A * C2 * 128].rearrange(
                "p (a c m) -> p a c m", a=A, c=C2); o += A * C2 * 128
            s_wdt = s_sbf[:, o : o + A]; o += A
            s_bbc = []
            for l in range(2):
                s_bbc.append(s_sbf[:, o : o + GPC * C8 * T].rearrange(
                    "p (g c t) -> p g c t", g=GPC, c=C8)); o += GPC * C8 * T
            s_blbc = s_sbf[:, o : o + GPC * C2 * T].rearrange(
                "p (g c t) -> p g c t", g=GPC, c=C2); o += GPC * C2 * T
            assert o == SBF_N
            o = 0
            s_b = []
            for l in range(2):
                s_b.append(s_sf32[:, o : o + GPC * C8].rearrange(
                    "p (g c) -> p g c", g=GPC)); o += GPC * C8
            s_blin = s_sf32[:, o : o + GPC * C2].rearrange(
                "p (g c) -> p g c", g=GPC); o += GPC * C2
            s_b1 = s_sf32[:, o : o + C2]; o += C2
            s_b2 = s_sf32[:, o : o + C2]; o += C2
            s_bd = s_sf32[0:1, o : o + 1]; o += 1
            assert o == SF32_N

            # per-(layer, group) weight tiles so compute on group g only
            # waits on g's own DMA; layer-0 weights stream in first
            w_lg = [[None] * GPC for _ in range(2)]
            w_lin = [None] * GPC
            for l in range(2):
                for g in range(GPC):
                    w_lg[l][g] = wpool.tile(
                        [128, 2, A, C8, 128], bf16, tag=f"w{l}_{g}", name=f"w{l}_{g}"
                    )
                    nc.sync.dma_start(w_lg[l][g][:], d_w[l][:, g])
            for g in range(GPC):
                w_lin[g] = wpool.tile(
                    [128, A, C2, 128], bf16, tag=f"wlin_{g}", name=f"wlin_{g}"
                )
                nc.sync.dma_start(w_lin[g][:], d_wlin[:, g])

            # ---- persistent activation buffers ----
            NH = 4           # group-quarters for elementwise batching
            GH = GPC // NH   # groups per quarter
            hbuf = [
                [
                    apool.tile(
                        [128, GH, A, T], bf16, tag=f"hbuf{l}_{h}", name=f"hbuf{l}_{h}"
                    )
                    for h in range(NH)
                ]
                for l in range(2)
            ]
            fcbf = apool.tile([128, GPC, C2, T], bf16, tag="fcbf", name="fcbf")
            lin1bf = apool.tile([128, C2, GPC, T], bf16, tag="lin1bf", name="lin1bf")
            s_lin1 = apool.tile([128, C2, GPC, T], f32, tag="lin1", name="s_lin1")
            s_expl = apool.tile([128, C2, GPC, T], f32, tag="expl", name="s_expl")
            s_beta = apool.tile([128, C2, GPC, T], f32, tag="beta", name="s_beta")
            s_gamma = apool.tile([1, GPC, T], f32, tag="gamma", name="s_gamma")
            s_hn = apool.tile([128, 2, GPC, A], f32, tag="hn", name="s_hn")
            s_cn = apool.tile([128, 2, GPC, A], f32, tag="cn", name="s_cn")

            # Make the first ACT instruction a Sigmoid so the initial
            # activation-table install picks the sigmoid/tanh table (the
            # xproj copies below are in every table).
            warm = tpool.tile([1, 1], f32, tag="warm", name="warm")
            nc.gpsimd.memset(warm[:], 0.0)
            nc.scalar.activation(warm[:], warm[:], Sig)

            # ================= the two LSTM layers =================
            for l in range(2):
                # --- x-projection+bias for all groups/timesteps (bf16) ---
                # xp[p, t, g, c] = (W_ih x_t + bih + bhh)[perm c][128c+p]
                xp = apool.tile(
                    [128, GPC, C8, T], bf16, tag="xp", bufs=2, name=f"xp{l}"
                )
                for g in range(GPC):
                    ps = pspool.tile(
                        [128, C8, T], f32, tag="xp_ps", bufs=2, name=f"xps{l}_{g}"
                    )
                    # bias preload via identity matmul (one shot, whole group)
                    nc.tensor.matmul(
                        ps[:], s_ident[:], s_bbc[l][:, g], start=True, stop=True
                    )
                    for c in range(C8):
                        for a in range(A):
                            if l == 0:
                                rhs = s_xT[:, g, a, :]
                            else:
                                rhs = hbuf[0][g // GH][:, g % GH, a, :]
                            nc.tensor.matmul(
                                ps[:, c, :],
                                w_lg[l][g][:, 0, a, c, :],
                                rhs,
                                start=False,
                                stop=(a == 1),
                                skip_group_check=True,
                            )
                    # PSUM -> SBUF(bf16) on ScalarE (idle in this phase);
                    # contiguous out (strided ACT writes fault on HW)
                    nc.scalar.copy(xp[:, g], ps[:])

                # --- recurrence ---
                cst = [
                    apool.tile(
                        [128, GH, A], f32, tag=f"cst{l}_{h}", name=f"cst{l}_{h}"
                    )
                    for h in range(NH)
                ]
                # Layer 0's back quarters are staggered one timestep
                # behind in emission order so their matmuls don't
                # head-of-line-block the PE stream while their weight
                # DMAs are still in flight.
                lag = [0, 0, 1, 1] if l == 0 else [0, 0, 0, 0]
                for it in range(T + max(lag)):
                    work = [
                        (it - lag[h], h) for h in range(NH)
                        if 0 <= it - lag[h] < T
                    ]
                    pss = {}
                    for t, h in work:
                        g0 = h * GH
                        ps = pspool.tile(
                            [128, GH, C8], f32, tag="gate_ps", bufs=3,
                            name=f"gps{l}_{t}_{h}",
                        )
                        # preload xp(+bias) into the gates PSUM region.
                        # stop=True closes the sim's accumulation group; the
                        # recurrence matmuls below keep accumulating on top
                        # (hardware per-element has_written semantics) with
                        # skip_group_check.
                        nc.tensor.matmul(
                            ps[:],
                            s_ident[:],
                            xp[:, g0 : g0 + GH, :, t],
                            start=True,
                            stop=True,
                        )
                        if t > 0:
                            for gi in range(GH):
                                g = g0 + gi
                                for c in range(C8):
                                    for a in range(A):
                                        nc.tensor.matmul(
                                            ps[:, gi, c : c + 1],
                                            w_lg[l][g][:, 1, a, c, :],
                                            hbuf[l][h][:, g % GH, a, t - 1 : t],
                                            start=False,
                                            stop=(a == 1),
                                            skip_group_check=True,
                                        )
                        pss[h] = ps

                    for t, h in work:
                        g0 = h * GH
                        ps = pss[h]
                        # gates are complete in PSUM; ACT reads PSUM direct.
                        # one sigmoid covers i,f,o AND the 2x-prescaled g
                        # (tanh(g) = 2*sigmoid(2g)-1, finished on DVE).
                        sif = tpool.tile(
                            [128, GH, 8], f32, tag="sif", bufs=6, name=f"sif{l}{t}{h}"
                        )
                        nc.scalar.activation(sif[:], ps[:], Sig)
                        tg = tpool.tile(
                            [128, GH, A], f32, tag="tg", bufs=6, name=f"tg{l}{t}{h}"
                        )
                        nc.gpsimd.tensor_scalar(
                            tg[:], sif[:, :, 6:8], 2.0, -1.0,
                            mybir.AluOpType.mult, mybir.AluOpType.add,
                        )

                        if t == 0:
                            # c = i * g
                            nc.vector.tensor_mul(cst[h][:], sif[:, :, 0:2], tg[:])
                        else:
                            fmul = tpool.tile(
                                [128, GH, A], f32, tag="fmul", bufs=6,
                                name=f"fm{l}{t}{h}",
                            )
                            nc.gpsimd.tensor_mul(fmul[:], sif[:, :, 2:4], cst[h][:])
                            ig = tpool.tile(
                                [128, GH, A], f32, tag="ig", bufs=6, name=f"ig{l}{t}{h}"
                            )
                            nc.vector.tensor_mul(ig[:], sif[:, :, 0:2], tg[:])
                            nc.vector.tensor_add(cst[h][:], fmul[:], ig[:])
                        tch = tpool.tile(
                            [128, GH, A], f32, tag="tch", bufs=6, name=f"tc{l}{t}{h}"
                        )
                        nc.scalar.activation(tch[:], cst[h][:], Tanh)
                        # h_t (bf16) into the sequence buffer
                        nc.vector.tensor_mul(
                            hbuf[l][h][:, :, :, t], sif[:, :, 4:6], tch[:]
                        )
                        if t == T - 1:
                            nc.vector.tensor_mul(
                                s_hn[:, l, g0 : g0 + GH, :], sif[:, :, 4:6], tch[:]
                            )
                            nc.vector.tensor_copy(
                                s_cn[:, l, g0 : g0 + GH, :], cst[h][:]
                            )

            # Dummy Exp so the exp/ln activation table loads during the
            # ScalarE idle window after the recurrence, not on the heads'
            # critical path. Reading s_cn (finished at t=T-1 of layer 1)
            # pins it after the last recurrence sigmoid/tanh.
            nc.scalar.activation(warm[:], s_cn[0:1, 0:1, 0:1, 0:1], Exp)

            # ================= per-group Linear =================
            fps = pspool.tile(
                [128, GPC, C2, T], f32, tag="fc_ps", bufs=1, name="fcps"
            )
            nc.tensor.matmul(
                fps[:], s_ident[:], s_blbc[:], start=True, stop=True
            )
            for g in range(GPC):
                for c2 in range(C2):
                    for a in range(A):
                        nc.tensor.matmul(
                            fps[:, g, c2, :],
                            w_lin[g][:, a, c2, :],
                            hbuf[1][g // GH][:, g % GH, a, :],
                            start=False,
                            stop=(a == 1),
                            skip_group_check=True,
                        )
            nc.scalar.copy(fcbf[:], fps[:])

            # ================= shared heads (batched over groups) =========
            # softplus = ln(1 + exp(x)): all Exp ACTs, then one Ln ACT,
            # so the scalar engine switches activation tables only once.
            psAs = []
            for c2 in range(C2):
                psA = pspool.tile(
                    [128, GPC * T], f32, tag="head", bufs=2, name=f"psA{c2}"
                )
                for a in range(A):
                    nc.tensor.matmul(
                        psA[:],
                        s_w1t[:, a, c2, :],
                        fcbf[:, :, a, :],
                        start=(a == 0),
                        stop=(a == 1),
                    )
                nc.vector.tensor_scalar_add(
                    s_lin1[:, c2].rearrange("p g t -> p (g t)"),
                    psA[:],
                    s_b1[:, c2 : c2 + 1],
                )
                nc.vector.tensor_scalar_add(
                    lin1bf[:, c2].rearrange("p g t -> p (g t)"),
                    psA[:],
                    s_b1[:, c2 : c2 + 1],
                )
                nc.scalar.activation(
                    s_expl[:, c2].rearrange("p g t -> p (g t)"),
                    psA[:],
                    Exp,
                    bias=s_b1[:, c2 : c2 + 1],
                )
                psAs.append(psA)
            betaexp = tpool.tile(
                [128, C2, GPC * T], f32, tag="betaexp", name="betaexp"
            )
            for c2 in range(C2):
                psB = pspool.tile(
                    [128, GPC * T], f32, tag="head", bufs=2, name=f"psB{c2}"
                )
                for a in range(A):
                    nc.tensor.matmul(
                        psB[:],
                        s_w2t[:, a, c2, :],
                        fcbf[:, :, a, :],
                        start=(a == 0),
                        stop=(a == 1),
                    )
                nc.scalar.activation(
                    betaexp[:, c2, :], psB[:], Exp, bias=s_b2[:, c2 : c2 + 1]
                )
            nc.scalar.activation(
                s_beta.rearrange("p c g t -> p (c g t)"),
                betaexp.rearrange("p c n -> p (c n)"),
                Ln,
                bias=1.0,
            )

            psG = pspool.tile([1, GPC * T], f32, tag="head", bufs=2, name="psG")
            for a in range(A):
                nc.tensor.matmul(
                    psG[:],
                    s_wdt[:, a : a + 1],
                    lin1bf[:, a].rearrange("p g t -> p (g t)"),
                    start=(a == 0),
                    stop=(a == 1),
                )
            nc.vector.tensor_scalar_add(
                s_gamma.rearrange("p g t -> p (g t)"), psG[:], s_bd[:, 0:1]
            )

            # ---- outputs ----
            nc.sync.dma_start(d_hn[:], s_hn[:])
            nc.sync.dma_start(d_cn[:], s_cn[:])
            nc.sync.dma_start(d_lin1[:], s_lin1[:])
            nc.sync.dma_start(d_expl[:], s_expl[:])
            nc.sync.dma_start(d_gamma[:], s_gamma[:])
            nc.sync.dma_start(d_beta[:], s_beta[:])

    nc.compile()
    return nc


def _prep_core_inputs(core, data, Wih0, Whh0, bih0, bhh0, Wih1, Whh1, bih1, bhh1,
                      Wlin, blin, W1, b1, W2, b2, Wd, bd):
    """Host-side shard + retile + bf16 cast for one core."""
    lo, hi = core * GPC, (core + 1) * GPC
    # PyTorch gate order is (i,f,g,o) in 256-blocks = chunks [i,i,f,f,g,g,o,o];
    # the device wants [i,i,f,f,o,o,g,g] (contiguous sigmoid slice).
    GPERM = np.array([0, 1, 2, 3, 6, 7, 4, 5])

    def wtiles(W, cdim):
        # W: [GPC, cdim*128, 256] -> [128(q), GPC, A, cdim, 128(m)]
        arr = W.reshape(GPC, cdim, 128, A, 128)  # [g, c, m, a, q]
        if cdim == C8:
            arr = arr[:, GPERM].copy()
            # tanh(g) is computed as 2*sigmoid(2*g)-1; fold the 2x here
            arr[:, 6:8] *= 2.0
        return np.ascontiguousarray(arr.transpose(4, 0, 3, 1, 2)).astype(BF16)

    def btiles(b, cdim):
        # b: [GPC, cdim*128] -> [128(p), GPC, cdim]
        arr = b.reshape(GPC, cdim, 128)
        if cdim == C8:
            arr = arr[:, GPERM].copy()
            arr[:, 6:8] *= 2.0
        return np.ascontiguousarray(arr.transpose(2, 0, 1)).astype(np.float32)

    d = {}
    x = data[:, lo:hi, :]  # [T, GPC, 256]
    xT = np.ascontiguousarray(
        x.reshape(T, GPC, A, 128).transpose(3, 1, 2, 0)
    ).astype(BF16)
    d["w0"] = np.stack([wtiles(Wih0[lo:hi], C8), wtiles(Whh0[lo:hi], C8)], axis=2)
    d["w1"] = np.stack([wtiles(Wih1[lo:hi], C8), wtiles(Whh1[lo:hi], C8)], axis=2)
    d["wlin"] = wtiles(Wlin[lo:hi], C2)
    # shared heads: [C2*128, 256] -> [128(q), A, C2, 128(m)]
    wsh = {}
    for name, W in (("w1t", W1), ("w2t", W2)):
        arr = W.reshape(C2, 128, A, 128)  # [c2, m, a, q]
        wsh[name] = np.ascontiguousarray(arr.transpose(3, 2, 0, 1)).astype(BF16)
    bbc0 = np.broadcast_to(
        btiles(bih0[lo:hi] + bhh0[lo:hi], C8)[..., None], (128, GPC, C8, T)
    ).astype(BF16)
    bbc1 = np.broadcast_to(
        btiles(bih1[lo:hi] + bhh1[lo:hi], C8)[..., None], (128, GPC, C8, T)
    ).astype(BF16)
    d["sbf"] = np.concatenate(
        [
            xT.reshape(128, -1),
            np.eye(128, dtype=BF16),
            wsh["w1t"].reshape(128, -1),
            wsh["w2t"].reshape(128, -1),
            np.ascontiguousarray(Wd.reshape(A, 128).T).astype(BF16),
            bbc0.reshape(128, -1),
            bbc1.reshape(128, -1),
            np.ascontiguousarray(np.broadcast_to(
                btiles(blin[lo:hi], C2)[..., None], (128, GPC, C2, T)
            )).astype(BF16).reshape(128, -1),
        ],
        axis=1,
    )
    bdcol = np.zeros((128, 1), np.float32)
    bdcol[0, 0] = float(np.asarray(bd).reshape(-1)[0])
    d["sf32"] = np.concatenate(
        [
            btiles(bih0[lo:hi] + bhh0[lo:hi], C8).reshape(128, -1),
            btiles(bih1[lo:hi] + bhh1[lo:hi], C8).reshape(128, -1),
            btiles(blin[lo:hi], C2).reshape(128, -1),
            np.ascontiguousarray(b1.reshape(C2, 128).T).astype(np.float32),
            np.ascontiguousarray(b2.reshape(C2, 128).T).astype(np.float32),
            bdcol,
        ],
        axis=1,
    )
    assert d["sbf"].shape[1] == SBF_N and d["sf32"].shape[1] == SF32_N
    return d


def _get_compiled():
    if "nc" not in _COMPILED:
        _COMPILED["nc"] = _build_nc()
    return _COMPILED["nc"]


def run_device(in_maps, trace=False, tmpdir=None):
    from concourse import bass_utils

    nc = _get_compiled()
    kw = {}
    if trace:
        kw = dict(trace=True, tmpdir=tmpdir)
    res = bass_utils.run_bass_kernel_spmd(
        nc, in_maps, core_ids=list(range(NCORES)), **kw
    )
    return res


def assemble(results):
    """Per-core device outputs -> full reference-shaped outputs."""
    lin1 = np.empty((G, T, H), np.float32)
    expl = np.empty((G, T, H), np.float32)
    beta = np.empty((G, T, H), np.float32)
    gamma = np.empty((G, T, 1), np.float32)
    hN = np.empty((G, 2, H), np.float32)
    cN = np.empty((G, 2, H), np.float32)
    for core in range(NCORES):
        r = results[core]
        lo = core * GPC
        # [128(p), C2, GPC, T] -> [g, t, c2*128+p]
        for name, dst in (("lin1o", lin1), ("explo", expl), ("betao", beta)):
            v = r[name].reshape(128, C2, GPC, T)
            dst[lo : lo + GPC] = v.transpose(2, 3, 1, 0).reshape(GPC, T, H)
        gamma[lo : lo + GPC] = r["gammao"].reshape(GPC, T, 1)
        # [128(p), l, g, a] -> [g, l, a*128+p]
        for name, dst in (("hno", hN), ("cno", cN)):
            v = r[name].reshape(128, 2, GPC, A)
            dst[lo : lo + GPC] = v.transpose(2, 1, 3, 0).reshape(GPC, 2, H)
    delta = expl / expl.sum(axis=0, keepdims=True)
    return gamma, beta, delta, hN, cN


def kernel(**inputs):
    in_maps = [_prep_core_inputs(c, **inputs) for c in range(NCORES)]
    res = run_device(in_maps)
    return assemble(res.results)


# revision 41
# speedup vs baseline: 1.0242x; 1.0242x over previous
"""Bass/Trainium2 kernel for nn_Encoder_32452772888844.

64 independent 2-layer LSTM(256) encoders + per-group Linear(256,256),
then shared heads:
  lin1  = fc @ W1.T + b1
  delta = softmax(lin1, axis=0)   (over the 64 groups)
  beta  = softplus(fc @ W2.T + b2)
  gamma = lin1 @ Wd.T + bd
Sharding: pure group parallelism — 8 groups per NeuronCore; each core
computes everything for its groups including exp(lin1); the softmax
normalization (a sum over the 64-group axis) is applied on the host.

Device-side formulation (per core):
  - All matmuls are weight-stationary: lhsT tiles [K=128, M=128] are
    (transposed) weight blocks, the moving operand is the activation
    vector/sequence. Gate results land in PSUM with the hidden dim on
    partitions, which makes the LSTM cell elementwise work efficient.
  - Weights are cast to bf16 on the host (PSUM accumulates in fp32).
  - x-projections (with biases) for all 10 timesteps are batched up
    front per layer; at each recurrence step an identity matmul
    preloads x-projection+bias into the gates PSUM region and the
    h-recurrence matmuls accumulate on top, so the ScalarE activations
    read gates straight out of PSUM (no elementwise add on the
    critical path).
  - Groups are processed in two halves per step so one half's cell
    math overlaps the other half's matmuls.
  - Gate chunk order is host-permuted from PyTorch's (i,f,g,o) to
    (i,i,f,f,o,o,g,g) 128-blocks so sigmoid covers one contiguous
    slice.
"""

import numpy as np
import ml_dtypes

T = 10
IN = 256
H = 256
G = 64
NCORES = 8
GPC = G // NCORES  # groups per core
A = 2   # 128-halves of 256
C8 = 8  # 128-chunks of 1024
C2 = 2  # 128-chunks of 256

BF16 = ml_dtypes.bfloat16
SBF_N = GPC * A * T + 128 + 2 * (A * C2 * 128) + A + 2 * GPC * C8 * T + GPC * C2 * T
SF32_N = 2 * GPC * C8 + GPC * C2 + C2 + C2 + 1

_COMPILED = {}


def _build_nc():
    import concourse.tile as tile
    from concourse import bacc, mybir

    f32 = mybir.dt.float32
    bf16 = mybir.dt.bfloat16
    Sig = mybir.ActivationFunctionType.Sigmoid
    Tanh = mybir.ActivationFunctionType.Tanh
    Exp = mybir.ActivationFunctionType.Exp
    Ln = mybir.ActivationFunctionType.Ln

    nc = bacc.Bacc(None, target_bir_lowering=False)

    # ---- DRAM parameters (per-core shards, host-prepared layouts) ----
    d_sbf = nc.dram_tensor("sbf", [128, SBF_N], bf16, kind="ExternalInput")
    d_sf32 = nc.dram_tensor("sf32", [128, SF32_N], f32, kind="ExternalInput")
    # wih and whh merged: dim1 -> 0=ih, 1=hh
    d_w = [
        nc.dram_tensor("w0", [128, GPC, 2, A, C8, 128], bf16, kind="ExternalInput"),
        nc.dram_tensor("w1", [128, GPC, 2, A, C8, 128], bf16, kind="ExternalInput"),
    ]
    d_wlin = nc.dram_tensor("wlin", [128, GPC, A, C2, 128], bf16, kind="ExternalInput")

    d_lin1 = nc.dram_tensor("lin1o", [128, C2, GPC, T], f32, kind="ExternalOutput")
    d_expl = nc.dram_tensor("explo", [128, C2, GPC, T], f32, kind="ExternalOutput")
    d_beta = nc.dram_tensor("betao", [128, C2, GPC, T], f32, kind="ExternalOutput")
    d_gamma = nc.dram_tensor("gammao", [1, GPC, T], f32, kind="ExternalOutput")
    d_hn = nc.dram_tensor("hno", [128, 2, GPC, A], f32, kind="ExternalOutput")
    d_cn = nc.dram_tensor("cno", [128, 2, GPC, A], f32, kind="ExternalOutput")

    with tile.TileContext(nc) as tc:
        with (
            tc.tile_pool(name="wpool", bufs=1) as wpool,
            tc.tile_pool(name="apool", bufs=1) as apool,
            tc.tile_pool(name="tpool", bufs=1) as tpool,
            tc.tile_pool(name="pspool", bufs=1, space="PSUM") as pspool,
        ):
            # ---- loads, in compute-consumption order ----
            # All small tensors are host-packed into two tensors (one per
            # dtype) so only two DMA-issue slots sit ahead of the weights.
            s_sbf = apool.tile([128, SBF_N], bf16, tag="sbf", name="s_sbf")
            nc.sync.dma_start(s_sbf[:], d_sbf[:])
            s_sf32 = apool.tile([128, SF32_N], f32, tag="sf32", name="s_sf32")
            nc.sync.dma_start(s_sf32[:], d_sf32[:])
            o = 0
            s_xT = s_sbf[:, o : o + GPC * A * T].rearrange(
                "p (g a t) -> p g a t", g=GPC, a=A); o += GPC * A * T
            s_ident = s_sbf[:, o : o + 128]; o += 128
            s_w1t = s_sbf[:, o : o + A * C2 * 128].rearrange(
                "p (a c m) -> p a c m", a=A, c=C2); o += A * C2 * 128
            s_w2t = s_sbf[:, o : o + A * C2 * 128].rearrange(
                "p (a c m) -> p a c m", a=A, c=C2); o += A * C2 * 128
            s_wdt = s_sbf[:, o : o + A]; o += A
            s_bbc = []
            for l in range(2):
                s_bbc.append(s_sbf[:, o : o + GPC * C8 * T].rearrange(
                    "p (g c t) -> p g c t", g=GPC, c=C8)); o += GPC * C8 * T
            s_blbc = s_sbf[:, o : o + GPC * C2 * T].rearrange(
                "p (g c t) -> p g c t", g=GPC, c=C2); o += GPC * C2 * T
            assert o == SBF_N
            o = 0
            s_b = []
            for l in range(2):
                s_b.append(s_sf32[:, o : o + GPC * C8].rearrange(
                    "p (g c) -> p g c", g=GPC)); o += GPC * C8
            s_blin = s_sf32[:, o : o + GPC * C2].rearrange(
                "p (g c) -> p g c", g=GPC); o += GPC * C2
            s_b1 = s_sf32[:, o : o + C2]; o += C2
            s_b2 = s_sf32[:, o : o + C2]; o += C2
            s_bd = s_sf32[0:1, o : o + 1]; o += 1
            assert o == SF32_N

            # per-(layer, group) weight tiles so compute on group g only
            # waits on g's own DMA; layer-0 weights stream in first
            w_lg = [[None] * GPC for _ in range(2)]
            w_lin = [None] * GPC
            for l in range(2):
                for g in range(GPC):
                    w_lg[l][g] = wpool.tile(
                        [128, 2, A, C8, 128], bf16, tag=f"w{l}_{g}", name=f"w{l}_{g}"
                    )
                    nc.sync.dma_start(w_lg[l][g][:], d_w[l][:, g])
            for g in range(GPC):
                w_lin[g] = wpool.tile(
                    [128, A, C2, 128], bf16, tag=f"wlin_{g}", name=f"wlin_{g}"
                )
                nc.sync.dma_start(w_lin[g][:], d_wlin[:, g])

            # ---- persistent activation buffers ----
            NH = 4           # group-quarters for elementwise batching
            GH = GPC // NH   # groups per quarter
            hbuf = [
                [
                    apool.tile(
                        [128, GH, A, T], bf16, tag=f"hbuf{l}_{h}", name=f"hbuf{l}_{h}"
                    )
                    for h in range(NH)
                ]
                for l in range(2)
            ]
            fcbf = apool.tile([128, GPC, C2, T], bf16, tag="fcbf", name="fcbf")
            lin1bf = apool.tile([128, C2, GPC, T], bf16, tag="lin1bf", name="lin1bf")
            s_lin1 = apool.tile([128, C2, GPC, T], f32, tag="lin1", name="s_lin1")
            s_expl = apool.tile([128, C2, GPC, T], f32, tag="expl", name="s_expl")
            s_beta = apool.tile([128, C2, GPC, T], f32, tag="beta", name="s_beta")
            s_gamma = apool.tile([1, GPC, T], f32, tag="gamma", name="s_gamma")
            s_hn = apool.tile([128, 2, GPC, A], f32, tag="hn", name="s_hn")
            s_cn = apool.tile([128, 2, GPC, A], f32, tag="cn", name="s_cn")

            # Make the first ACT instruction a Sigmoid so the initial
            # activation-table install picks the sigmoid/tanh table (the
            # xproj copies below are in every table).
            warm = tpool.tile([1, 1], f32, tag="warm", name="warm")
            nc.gpsimd.memset(warm[:], 0.0)
            nc.scalar.activation(warm[:], warm[:], Sig)

            # ================= the two LSTM layers =================
            for l in range(2):
                # --- x-projection+bias for all groups/timesteps (bf16) ---
                # xp[p, t, g, c] = (W_ih x_t + bih + bhh)[perm c][128c+p]
                xp = apool.tile(
                    [128, GPC, C8, T], bf16, tag="xp", bufs=2, name=f"xp{l}"
                )
                for g in range(GPC):
                    ps = pspool.tile(
                        [128, C8, T], f32, tag="xp_ps", bufs=2, name=f"xps{l}_{g}"
                    )
                    # bias preload via identity matmul (one shot, whole group)
                    nc.tensor.matmul(
                        ps[:], s_ident[:], s_bbc[l][:, g], start=True, stop=True
                    )
                    for c in range(C8):
                        for a in range(A):
                            if l == 0:
                                rhs = s_xT[:, g, a, :]
                            else:
                                rhs = hbuf[0][g // GH][:, g % GH, a, :]
                            nc.tensor.matmul(
                                ps[:, c, :],
                                w_lg[l][g][:, 0, a, c, :],
                                rhs,
                                start=False,
                                stop=(a == 1),
                                skip_group_check=True,
                            )
                    # PSUM -> SBUF(bf16) on ScalarE (idle in this phase);
                    # contiguous out (strided ACT writes fault on HW)
                    nc.scalar.copy(xp[:, g], ps[:])

                # --- recurrence ---
                cst = [
                    apool.tile(
                        [128, GH, A], f32, tag=f"cst{l}_{h}", name=f"cst{l}_{h}"
                    )
                    for h in range(NH)
                ]
                # Layer 0's back quarters are staggered one timestep
                # behind in emission order so their matmuls don't
                # head-of-line-block the PE stream while their weight
                # DMAs are still in flight.
                lag = [0, 0, 1, 1] if l == 0 else [0, 0, 0, 0]
                for it in range(T + max(lag)):
                    work = [
                        (it - lag[h], h) for h in range(NH)
                        if 0 <= it - lag[h] < T
                    ]
                    pss = {}
                    for t, h in work:
                        g0 = h * GH
                        ps = pspool.tile(
                            [128, GH, C8], f32, tag="gate_ps", bufs=3,
                            name=f"gps{l}_{t}_{h}",
                        )
                        # preload xp(+bias) into the gates PSUM region.
                        # stop=True closes the sim's accumulation group; the
                        # recurrence matmuls below keep accumulating on top
                        # (hardware per-element has_written semantics) with
                        # skip_group_check.
                        nc.tensor.matmul(
                            ps[:],
                            s_ident[:],
                            xp[:, g0 : g0 + GH, :, t],
                            start=True,
                            stop=True,
                        )
                        if t > 0:
                            for gi in range(GH):
                                g = g0 + gi
                                for c in range(C8):
                                    for a in range(A):
                                        nc.tensor.matmul(
                                            ps[:, gi, c : c + 1],
                                            w_lg[l][g][:, 1, a, c, :],
                                            hbuf[l][h][:, g % GH, a, t - 1 : t],
                                            start=False,
                                            stop=(a == 1),
                                            skip_group_check=True,
                                        )
                        pss[h] = ps

                    for t, h in work:
                        g0 = h * GH
                        ps = pss[h]
                        # gates are complete in PSUM; ACT reads PSUM direct.
                        # one sigmoid covers i,f,o AND the 2x-prescaled g
                        # (tanh(g) = 2*sigmoid(2g)-1, finished on DVE).
                        sif = tpool.tile(
                            [128, GH, 8], f32, tag="sif", bufs=6, name=f"sif{l}{t}{h}"
                        )
                        nc.scalar.activation(sif[:], ps[:], Sig)
                        tg = tpool.tile(
                            [128, GH, A], f32, tag="tg", bufs=6, name=f"tg{l}{t}{h}"
                        )
                        nc.gpsimd.tensor_scalar(
                            tg[:], sif[:, :, 6:8], 2.0, -1.0,
                            mybir.AluOpType.mult, mybir.AluOpType.add,
                        )

                        if t == 0:
                            # c = i * g
                            nc.vector.tensor_mul(cst[h][:], sif[:, :, 0:2], tg[:])
                        else:
                            fmul = tpool.tile(
                                [128, GH, A], f32, tag="fmul", bufs=6,
                                name=f"fm{l}{t}{h}",
                            )
                            nc.gpsimd.tensor_mul(fmul[:], sif[:, :, 2:4], cst[h][:])
                            ig = tpool.tile(
                                [128, GH, A], f32, tag="ig", bufs=6, name=f"ig{l}{t}{h}"
                            )
                            nc.vector.tensor_mul(ig[:], sif[:, :, 0:2], tg[:])
                            nc.vector.tensor_add(cst[h][:], fmul[:], ig[:])
                        tch = tpool.tile(
                            [128, GH, A], f32, tag="tch", bufs=6, name=f"tc{l}{t}{h}"
                        )
                        nc.scalar.activation(tch[:], cst[h][:], Tanh)
                        # h_t (bf16) into the sequence buffer
                        nc.vector.tensor_mul(
                            hbuf[l][h][:, :, :, t], sif[:, :, 4:6], tch[:]
                        )
                        if t == T - 1:
                            nc.vector.tensor_mul(
                                s_hn[:, l, g0 : g0 + GH, :], sif[:, :, 4:6], tch[:]
                            )
                            nc.vector.tensor_copy(
                                s_cn[:, l, g0 : g0 + GH, :], cst[h][:]
                            )

            # ================= per-group Linear =================
            fps = pspool.tile(
                [128, GPC, C2, T], f32, tag="fc_ps", bufs=1, name="fcps"
            )
            nc.tensor.matmul(
                fps[:], s_ident[:], s_blbc[:], start=True, stop=True
            )
            for g in range(GPC):
                for c2 in range(C2):
                    for a in range(A):
                        nc.tensor.matmul(
                            fps[:, g, c2, :],
                            w_lin[g][:, a, c2, :],
                            hbuf[1][g // GH][:, g % GH, a, :],
                            start=False,
                            stop=(a == 1),
                            skip_group_check=True,
                        )
            nc.scalar.copy(fcbf[:], fps[:])

            # ================= shared heads (batched over groups) =========
            # softplus = ln(1 + exp(x)): all Exp ACTs, then one Ln ACT,
            # so the scalar engine switches activation tables only once.
            psAs = []
            for c2 in range(C2):
                psA = pspool.tile(
                    [128, GPC * T], f32, tag="head", bufs=2, name=f"psA{c2}"
                )
                for a in range(A):
                    nc.tensor.matmul(
                        psA[:],
                        s_w1t[:, a, c2, :],
                        fcbf[:, :, a, :],
                        start=(a == 0),
                        stop=(a == 1),
                    )
                nc.vector.tensor_scalar_add(
                    s_lin1[:, c2].rearrange("p g t -> p (g t)"),
                    psA[:],
                    s_b1[:, c2 : c2 + 1],
                )
                nc.vector.tensor_scalar_add(
                    lin1bf[:, c2].rearrange("p g t -> p (g t)"),
                    psA[:],
                    s_b1[:, c2 : c2 + 1],
                )
                nc.scalar.activation(
                    s_expl[:, c2].rearrange("p g t -> p (g t)"),
                    psA[:],
                    Exp,
                    bias=s_b1[:, c2 : c2 + 1],
                )
                psAs.append(psA)
            betaexp = tpool.tile(
                [128, C2, GPC * T], f32, tag="betaexp", name="betaexp"
            )
            for c2 in range(C2):
                psB = pspool.tile(
                    [128, GPC * T], f32, tag="head", bufs=2, name=f"psB{c2}"
                )
                for a in range(A):
                    nc.tensor.matmul(
                        psB[:],
                        s_w2t[:, a, c2, :],
                        fcbf[:, :, a, :],
                        start=(a == 0),
                        stop=(a == 1),
                    )
                nc.scalar.activation(
                    betaexp[:, c2, :], psB[:], Exp, bias=s_b2[:, c2 : c2 + 1]
                )
            nc.scalar.activation(
                s_beta.rearrange("p c g t -> p (c g t)"),
                betaexp.rearrange("p c n -> p (c n)"),
                Ln,
                bias=1.0,
            )

            psG = pspool.tile([1, GPC * T], f32, tag="head", bufs=2, name="psG")
            for a in range(A):
                nc.tensor.matmul(
                    psG[:],
                    s_wdt[:, a : a + 1],
                    lin1bf[:, a].rearrange("p g t -> p (g t)"),
                    start=(a == 0),
                    stop=(a == 1),
                )
            nc.vector.tensor_scalar_add(
                s_gamma.rearrange("p g t -> p (g t)"), psG[:], s_bd[:, 0:1]
            )

            # ---- outputs ----
            nc.sync.dma_start(d_hn[:], s_hn[:])
            nc.sync.dma_start(d_cn[:], s_cn[:])
            nc.sync.dma_start(d_lin1[:], s_lin1[:])
            nc.sync.dma_start(d_expl[:], s_expl[:])
            nc.sync.dma_start(d_gamma[:], s_gamma[:])
            nc.sync.dma_start(d_beta[:], s_beta[:])

    nc.compile()
    return nc


def _prep_core_inputs(core, data, Wih0, Whh0, bih0, bhh0, Wih1, Whh1, bih1, bhh1,
                      Wlin, blin, W1, b1, W2, b2, Wd, bd):
    """Host-side shard + retile + bf16 cast for one core."""
    lo, hi = core * GPC, (core + 1) * GPC
    # PyTorch gate order is (i,f,g,o) in 256-blocks = chunks [i,i,f,f,g,g,o,o];
    # the device wants [i,i,f,f,o,o,g,g] (contiguous sigmoid slice).
    GPERM = np.array([0, 1, 2, 3, 6, 7, 4, 5])

    def wtiles(W, cdim):
        # W: [GPC, cdim*128, 256] -> [128(q), GPC, A, cdim, 128(m)]
        arr = W.reshape(GPC, cdim, 128, A, 128)  # [g, c, m, a, q]
        if cdim == C8:
            arr = arr[:, GPERM].copy()
            # tanh(g) is computed as 2*sigmoid(2*g)-1; fold the 2x here
            arr[:, 6:8] *= 2.0
        return np.ascontiguousarray(arr.transpose(4, 0, 3, 1, 2)).astype(BF16)

    def btiles(b, cdim):
        # b: [GPC, cdim*128] -> [128(p), GPC, cdim]
        arr = b.reshape(GPC, cdim, 128)
        if cdim == C8:
            arr = arr[:, GPERM].copy()
            arr[:, 6:8] *= 2.0
        return np.ascontiguousarray(arr.transpose(2, 0, 1)).astype(np.float32)

    d = {}
    x = data[:, lo:hi, :]  # [T, GPC, 256]
    xT = np.ascontiguousarray(
        x.reshape(T, GPC, A, 128).transpose(3, 1, 2, 0)
    ).astype(BF16)
    d["w0"] = np.stack([wtiles(Wih0[lo:hi], C8), wtiles(Whh0[lo:hi], C8)], axis=2)
    d["w1"] = np.stack([wtiles(Wih1[lo:hi], C8), wtiles(Whh1[lo:hi], C8)], axis=2)
    d["wlin"] = wtiles(Wlin[lo:hi], C2)
    # shared heads: [C2*128, 256] -> [128(q), A, C2, 128(m)]
    wsh = {}
    for name, W in (("w1t", W1), ("w2t", W2)):
        arr = W.reshape(C2, 128, A, 128)  # [c2, m, a, q]
        wsh[name] = np.ascontiguousarray(arr.transpose(3, 2, 0, 1)).astype(BF16)
    bbc0 = np.broadcast_to(
        btiles(bih0[lo:hi] + bhh0[lo:hi], C8)[..., None], (128, GPC, C8, T)
    ).astype(BF16)
    bbc1 = np.broadcast_to(
        btiles(bih1[lo:hi] + bhh1[lo:hi], C8)[..., None], (128, GPC, C8, T)
    ).astype(BF16)
    d["sbf"] = np.concatenate(
        [
            xT.reshape(128, -1),
            np.eye(128, dtype=BF16),
            wsh["w1t"].reshape(128, -1),
            wsh["w2t"].reshape(128, -1),
            np.ascontiguousarray(Wd.reshape(A, 128).T).astype(BF16),
            bbc0.reshape(128, -1),
            bbc1.reshape(128, -1),
            np.ascontiguousarray(np.broadcast_to(
                btiles(blin[lo:hi], C2)[..., None], (128, GPC, C2, T)
            )).astype(BF16).reshape(128, -1),
        ],
        axis=1,
    )
    bdcol = np.zeros((128, 1), np.float32)
    bdcol[0, 0] = float(np.asarray(bd).reshape(-1)[0])
    d["sf32"] = np.concatenate(
        [
            btiles(bih0[lo:hi] + bhh0[lo:hi], C8).reshape(128, -1),
            btiles(bih1[lo:hi] + bhh1[lo:hi], C8).reshape(128, -1),
            btiles(blin[lo:hi], C2).reshape(128, -1),
            np.ascontiguousarray(b1.reshape(C2, 128).T).astype(np.float32),
            np.ascontiguousarray(b2.reshape(C2, 128).T).astype(np.float32),
            bdcol,
        ],
        axis=1,
    )
    assert d["sbf"].shape[1] == SBF_N and d["sf32"].shape[1] == SF32_N
    return d


def _get_compiled():
    if "nc" not in _COMPILED:
        _COMPILED["nc"] = _build_nc()
    return _COMPILED["nc"]


def run_device(in_maps, trace=False, tmpdir=None):
    from concourse import bass_utils

    nc = _get_compiled()
    kw = {}
    if trace:
        kw = dict(trace=True, tmpdir=tmpdir)
    res = bass_utils.run_bass_kernel_spmd(
        nc, in_maps, core_ids=list(range(NCORES)), **kw
    )
    return res


def assemble(results):
    """Per-core device outputs -> full reference-shaped outputs."""
    lin1 = np.empty((G, T, H), np.float32)
    expl = np.empty((G, T, H), np.float32)
    beta = np.empty((G, T, H), np.float32)
    gamma = np.empty((G, T, 1), np.float32)
    hN = np.empty((G, 2, H), np.float32)
    cN = np.empty((G, 2, H), np.float32)
    for core in range(NCORES):
        r = results[core]
        lo = core * GPC
        # [128(p), C2, GPC, T] -> [g, t, c2*128+p]
        for name, dst in (("lin1o", lin1), ("explo", expl), ("betao", beta)):
            v = r[name].reshape(128, C2, GPC, T)
            dst[lo : lo + GPC] = v.transpose(2, 3, 1, 0).reshape(GPC, T, H)
        gamma[lo : lo + GPC] = r["gammao"].reshape(GPC, T, 1)
        # [128(p), l, g, a] -> [g, l, a*128+p]
        for name, dst in (("hno", hN), ("cno", cN)):
            v = r[name].reshape(128, 2, GPC, A)
            dst[lo : lo + GPC] = v.transpose(2, 1, 3, 0).reshape(GPC, 2, H)
    delta = expl / expl.sum(axis=0, keepdims=True)
    return gamma, beta, delta, hN, cN


def kernel(**inputs):
    in_maps = [_prep_core_inputs(c, **inputs) for c in range(NCORES)]
    res = run_device(in_maps)
    return assemble(res.results)


# revision 44
# speedup vs baseline: 1.0266x; 1.0023x over previous
"""Bass/Trainium2 kernel for nn_Encoder_32452772888844.

64 independent 2-layer LSTM(256) encoders + per-group Linear(256,256),
then shared heads:
  lin1  = fc @ W1.T + b1
  delta = softmax(lin1, axis=0)   (over the 64 groups)
  beta  = softplus(fc @ W2.T + b2)
  gamma = lin1 @ Wd.T + bd
Sharding: pure group parallelism — 8 groups per NeuronCore; each core
computes everything for its groups including exp(lin1); the softmax
normalization (a sum over the 64-group axis) is applied on the host.

Device-side formulation (per core):
  - All matmuls are weight-stationary: lhsT tiles [K=128, M=128] are
    (transposed) weight blocks, the moving operand is the activation
    vector/sequence. Gate results land in PSUM with the hidden dim on
    partitions, which makes the LSTM cell elementwise work efficient.
  - Weights are cast to bf16 on the host (PSUM accumulates in fp32).
  - x-projections (with biases) for all 10 timesteps are batched up
    front per layer; at each recurrence step an identity matmul
    preloads x-projection+bias into the gates PSUM region and the
    h-recurrence matmuls accumulate on top, so the ScalarE activations
    read gates straight out of PSUM (no elementwise add on the
    critical path).
  - Groups are processed in two halves per step so one half's cell
    math overlaps the other half's matmuls.
  - Gate chunk order is host-permuted from PyTorch's (i,f,g,o) to
    (i,i,f,f,o,o,g,g) 128-blocks so sigmoid covers one contiguous
    slice.
"""

import numpy as np
import ml_dtypes

T = 10
IN = 256
H = 256
G = 64
NCORES = 8
GPC = G // NCORES  # groups per core
A = 2   # 128-halves of 256
C8 = 8  # 128-chunks of 1024
C2 = 2  # 128-chunks of 256

BF16 = ml_dtypes.bfloat16
SBF_N = GPC * A * T + 128 + 2 * (A * C2 * 128) + A + 2 * GPC * C8 * T + GPC * C2 * T
SF32_N = 2 * GPC * C8 + GPC * C2 + C2 + C2 + 1

_COMPILED = {}


def _build_nc():
    import concourse.tile as tile
    from concourse import bacc, mybir

    f32 = mybir.dt.float32
    bf16 = mybir.dt.bfloat16
    Sig = mybir.ActivationFunctionType.Sigmoid
    Tanh = mybir.ActivationFunctionType.Tanh
    Exp = mybir.ActivationFunctionType.Exp
    Ln = mybir.ActivationFunctionType.Ln

    nc = bacc.Bacc(None, target_bir_lowering=False)

    # ---- DRAM parameters (per-core shards, host-prepared layouts) ----
    d_sbf = nc.dram_tensor("sbf", [128, SBF_N], bf16, kind="ExternalInput")
    d_sf32 = nc.dram_tensor("sf32", [128, SF32_N], f32, kind="ExternalInput")
    # wih and whh merged: dim1 -> 0=ih, 1=hh
    d_w = [
        nc.dram_tensor("w0", [128, GPC, 2, A, C8, 128], bf16, kind="ExternalInput"),
        nc.dram_tensor("w1", [128, GPC, 2, A, C8, 128], bf16, kind="ExternalInput"),
    ]
    d_wlin = nc.dram_tensor("wlin", [128, GPC, A, C2, 128], bf16, kind="ExternalInput")

    d_lin1 = nc.dram_tensor("lin1o", [128, C2, GPC, T], f32, kind="ExternalOutput")
    d_expl = nc.dram_tensor("explo", [128, C2, GPC, T], f32, kind="ExternalOutput")
    d_beta = nc.dram_tensor("betao", [128, C2, GPC, T], f32, kind="ExternalOutput")
    d_gamma = nc.dram_tensor("gammao", [1, GPC, T], f32, kind="ExternalOutput")
    d_hn = nc.dram_tensor("hno", [128, 2, GPC, A], f32, kind="ExternalOutput")
    d_cn = nc.dram_tensor("cno", [128, 2, GPC, A], f32, kind="ExternalOutput")

    with tile.TileContext(nc) as tc:
        with (
            tc.tile_pool(name="wpool", bufs=1) as wpool,
            tc.tile_pool(name="apool", bufs=1) as apool,
            tc.tile_pool(name="tpool", bufs=1) as tpool,
            tc.tile_pool(name="pspool", bufs=1, space="PSUM") as pspool,
        ):
            # ---- loads, in compute-consumption order ----
            # All small tensors are host-packed into two tensors (one per
            # dtype) so only two DMA-issue slots sit ahead of the weights.
            s_sbf = apool.tile([128, SBF_N], bf16, tag="sbf", name="s_sbf")
            nc.sync.dma_start(s_sbf[:], d_sbf[:])
            s_sf32 = apool.tile([128, SF32_N], f32, tag="sf32", name="s_sf32")
            nc.sync.dma_start(s_sf32[:], d_sf32[:])
            o = 0
            s_xT = s_sbf[:, o : o + GPC * A * T].rearrange(
                "p (g a t) -> p g a t", g=GPC, a=A); o += GPC * A * T
            s_ident = s_sbf[:, o : o + 128]; o += 128
            s_w1t = s_sbf[:, o : o + A * C2 * 128].rearrange(
                "p (a c m) -> p a c m", a=A, c=C2); o += A * C2 * 128
            s_w2t = s_sbf[:, o : o + A * C2 * 128].rearrange(
                "p (a c m) -> p a c m", a=A, c=C2); o += A * C2 * 128
            s_wdt = s_sbf[:, o : o + A]; o += A
            s_bbc = []
            for l in range(2):
                s_bbc.append(s_sbf[:, o : o + GPC * C8 * T].rearrange(
                    "p (g c t) -> p g c t", g=GPC, c=C8)); o += GPC * C8 * T
            s_blbc = s_sbf[:, o : o + GPC * C2 * T].rearrange(
                "p (g c t) -> p g c t", g=GPC, c=C2); o += GPC * C2 * T
            assert o == SBF_N
            o = 0
            s_b = []
            for l in range(2):
                s_b.append(s_sf32[:, o : o + GPC * C8].rearrange(
                    "p (g c) -> p g c", g=GPC)); o += GPC * C8
            s_blin = s_sf32[:, o : o + GPC * C2].rearrange(
                "p (g c) -> p g c", g=GPC); o += GPC * C2
            s_b1 = s_sf32[:, o : o + C2]; o += C2
            s_b2 = s_sf32[:, o : o + C2]; o += C2
            s_bd = s_sf32[0:1, o : o + 1]; o += 1
            assert o == SF32_N

            # per-(layer, group) weight tiles so compute on group g only
            # waits on g's own DMA; layer-0 weights stream in first
            w_lg = [[None] * GPC for _ in range(2)]
            w_lin = [None] * GPC
            for l in range(2):
                for g in range(GPC):
                    w_lg[l][g] = wpool.tile(
                        [128, 2, A, C8, 128], bf16, tag=f"w{l}_{g}", name=f"w{l}_{g}"
                    )
                    nc.sync.dma_start(w_lg[l][g][:], d_w[l][:, g])
            for g in range(GPC):
                w_lin[g] = wpool.tile(
                    [128, A, C2, 128], bf16, tag=f"wlin_{g}", name=f"wlin_{g}"
                )
                nc.sync.dma_start(w_lin[g][:], d_wlin[:, g])

            # ---- persistent activation buffers ----
            NH = 4           # group-quarters for elementwise batching
            GH = GPC // NH   # groups per quarter
            hbuf = [
                [
                    apool.tile(
                        [128, GH, A, T], bf16, tag=f"hbuf{l}_{h}", name=f"hbuf{l}_{h}"
                    )
                    for h in range(NH)
                ]
                for l in range(2)
            ]
            fcbf = apool.tile([128, GPC, C2, T], bf16, tag="fcbf", name="fcbf")
            lin1bf = apool.tile([128, C2, GPC, T], bf16, tag="lin1bf", name="lin1bf")
            s_lin1 = apool.tile([128, C2, GPC, T], f32, tag="lin1", name="s_lin1")
            s_expl = apool.tile([128, C2, GPC, T], f32, tag="expl", name="s_expl")
            s_beta = apool.tile([128, C2, GPC, T], f32, tag="beta", name="s_beta")
            s_gamma = apool.tile([1, GPC, T], f32, tag="gamma", name="s_gamma")
            s_hn = apool.tile([128, 2, GPC, A], f32, tag="hn", name="s_hn")
            s_cn = apool.tile([128, 2, GPC, A], f32, tag="cn", name="s_cn")

            # Make the first ACT instruction a Sigmoid so the initial
            # activation-table install picks the sigmoid/tanh table (the
            # xproj copies below are in every table).
            warm = tpool.tile([1, 1], f32, tag="warm", name="warm")
            nc.gpsimd.memset(warm[:], 0.0)
            nc.scalar.activation(warm[:], warm[:], Sig)

            # ================= the two LSTM layers =================
            for l in range(2):
                # --- x-projection+bias for all groups/timesteps (bf16) ---
                # xp[p, t, g, c] = (W_ih x_t + bih + bhh)[perm c][128c+p]
                xp = apool.tile(
                    [128, GPC, C8, T], bf16, tag="xp", bufs=2, name=f"xp{l}"
                )
                for g in range(GPC):
                    ps = pspool.tile(
                        [128, C8, T], f32, tag="head", bufs=2, name=f"xps{l}_{g}"
                    )
                    # bias preload via identity matmul (one shot, whole group)
                    nc.tensor.matmul(
                        ps[:], s_ident[:], s_bbc[l][:, g], start=True, stop=True
                    )
                    for c in range(C8):
                        for a in range(A):
                            if l == 0:
                                rhs = s_xT[:, g, a, :]
                            else:
                                rhs = hbuf[0][g // GH][:, g % GH, a, :]
                            nc.tensor.matmul(
                                ps[:, c, :],
                                w_lg[l][g][:, 0, a, c, :],
                                rhs,
                                start=False,
                                stop=(a == 1),
                                skip_group_check=True,
                            )
                    # PSUM -> SBUF(bf16) on ScalarE (idle in this phase);
                    # contiguous out (strided ACT writes fault on HW)
                    nc.scalar.copy(xp[:, g], ps[:])

                # --- recurrence ---
                cst = [
                    apool.tile(
                        [128, GH, A], f32, tag=f"cst{l}_{h}", name=f"cst{l}_{h}"
                    )
                    for h in range(NH)
                ]
                # Layer 0's back quarters are staggered one timestep
                # behind in emission order so their matmuls don't
                # head-of-line-block the PE stream while their weight
                # DMAs are still in flight.
                lag = [0, 0, 1, 1] if l == 0 else [0, 0, 0, 0]
                for it in range(T + max(lag)):
                    work = [
                        (it - lag[h], h) for h in range(NH)
                        if 0 <= it - lag[h] < T
                    ]
                    pss = {}
                    for t, h in work:
                        g0 = h * GH
                        ps = pspool.tile(
                            [128, GH, C8], f32, tag="gate_ps", bufs=4,
                            name=f"gps{l}_{t}_{h}",
                        )
                        # preload xp(+bias) into the gates PSUM region.
                        # stop=True closes the sim's accumulation group; the
                        # recurrence matmuls below keep accumulating on top
                        # (hardware per-element has_written semantics) with
                        # skip_group_check.
                        nc.tensor.matmul(
                            ps[:],
                            s_ident[:],
                            xp[:, g0 : g0 + GH, :, t],
                            start=True,
                            stop=True,
                        )
                        if t > 0:
                            for gi in range(GH):
                                g = g0 + gi
                                for c in range(C8):
                                    for a in range(A):
                                        nc.tensor.matmul(
                                            ps[:, gi, c : c + 1],
                                            w_lg[l][g][:, 1, a, c, :],
                                            hbuf[l][h][:, g % GH, a, t - 1 : t],
                                            start=False,
                                            stop=(a == 1),
                                            skip_group_check=True,
                                        )
                        pss[h] = ps

                    for t, h in work:
                        g0 = h * GH
                        ps = pss[h]
                        # gates are complete in PSUM; ACT reads PSUM direct.
                        # one sigmoid covers i,f,o AND the 2x-prescaled g
                        # (tanh(g) = 2*sigmoid(2g)-1, finished on DVE).
                        sif = tpool.tile(
                            [128, GH, 8], f32, tag="sif", bufs=6, name=f"sif{l}{t}{h}"
                        )
                        nc.scalar.activation(sif[:], ps[:], Sig)
                        tg = tpool.tile(
                            [128, GH, A], f32, tag="tg", bufs=6, name=f"tg{l}{t}{h}"
                        )
                        nc.gpsimd.tensor_scalar(
                            tg[:], sif[:, :, 6:8], 2.0, -1.0,
                            mybir.AluOpType.mult, mybir.AluOpType.add,
                        )

                        if t == 0:
                            # c = i * g
                            nc.vector.tensor_mul(cst[h][:], sif[:, :, 0:2], tg[:])
                        else:
                            fmul = tpool.tile(
                                [128, GH, A], f32, tag="fmul", bufs=6,
                                name=f"fm{l}{t}{h}",
                            )
                            nc.gpsimd.tensor_mul(fmul[:], sif[:, :, 2:4], cst[h][:])
                            ig = tpool.tile(
                                [128, GH, A], f32, tag="ig", bufs=6, name=f"ig{l}{t}{h}"
                            )
                            nc.vector.tensor_mul(ig[:], sif[:, :, 0:2], tg[:])
                            nc.vector.tensor_add(cst[h][:], fmul[:], ig[:])
                        tch = tpool.tile(
                            [128, GH, A], f32, tag="tch", bufs=6, name=f"tc{l}{t}{h}"
                        )
                        nc.scalar.activation(tch[:], cst[h][:], Tanh)
                        # h_t (bf16) into the sequence buffer
                        nc.vector.tensor_mul(
                            hbuf[l][h][:, :, :, t], sif[:, :, 4:6], tch[:]
                        )
                        if t == T - 1:
                            nc.vector.tensor_mul(
                                s_hn[:, l, g0 : g0 + GH, :], sif[:, :, 4:6], tch[:]
                            )
                            nc.vector.tensor_copy(
                                s_cn[:, l, g0 : g0 + GH, :], cst[h][:]
                            )

            # ================= per-group Linear =================
            fps = pspool.tile(
                [128, GPC, C2, T], f32, tag="fc_ps", bufs=1, name="fcps"
            )
            nc.tensor.matmul(
                fps[:], s_ident[:], s_blbc[:], start=True, stop=True
            )
            for g in range(GPC):
                for c2 in range(C2):
                    for a in range(A):
                        nc.tensor.matmul(
                            fps[:, g, c2, :],
                            w_lin[g][:, a, c2, :],
                            hbuf[1][g // GH][:, g % GH, a, :],
                            start=False,
                            stop=(a == 1),
                            skip_group_check=True,
                        )
            nc.scalar.copy(fcbf[:], fps[:])

            # ================= shared heads (batched over groups) =========
            # softplus = ln(1 + exp(x)): all Exp ACTs, then one Ln ACT,
            # so the scalar engine switches activation tables only once.
            psAs = []
            for c2 in range(C2):
                psA = pspool.tile(
                    [128, GPC * T], f32, tag="head", bufs=2, name=f"psA{c2}"
                )
                for a in range(A):
                    nc.tensor.matmul(
                        psA[:],
                        s_w1t[:, a, c2, :],
                        fcbf[:, :, a, :],
                        start=(a == 0),
                        stop=(a == 1),
                    )
                nc.vector.tensor_scalar_add(
                    s_lin1[:, c2].rearrange("p g t -> p (g t)"),
                    psA[:],
                    s_b1[:, c2 : c2 + 1],
                )
                nc.vector.tensor_scalar_add(
                    lin1bf[:, c2].rearrange("p g t -> p (g t)"),
                    psA[:],
                    s_b1[:, c2 : c2 + 1],
                )
                nc.scalar.activation(
                    s_expl[:, c2].rearrange("p g t -> p (g t)"),
                    psA[:],
                    Exp,
                    bias=s_b1[:, c2 : c2 + 1],
                )
                psAs.append(psA)
            betaexp = tpool.tile(
                [128, C2, GPC * T], f32, tag="betaexp", name="betaexp"
            )
            for c2 in range(C2):
                psB = pspool.tile(
                    [128, GPC * T], f32, tag="head", bufs=2, name=f"psB{c2}"
                )
                for a in range(A):
                    nc.tensor.matmul(
                        psB[:],
                        s_w2t[:, a, c2, :],
                        fcbf[:, :, a, :],
                        start=(a == 0),
                        stop=(a == 1),
                    )
                nc.scalar.activation(
                    betaexp[:, c2, :], psB[:], Exp, bias=s_b2[:, c2 : c2 + 1]
                )
            nc.scalar.activation(
                s_beta.rearrange("p c g t -> p (c g t)"),
                betaexp.rearrange("p c n -> p (c n)"),
                Ln,
                bias=1.0,
            )

            psG = pspool.tile([1, GPC * T], f32, tag="head", bufs=2, name="psG")
            for a in range(A):
                nc.tensor.matmul(
                    psG[:],
                    s_wdt[:, a : a + 1],
                    lin1bf[:, a].rearrange("p g t -> p (g t)"),
                    start=(a == 0),
                    stop=(a == 1),
                )
            nc.vector.tensor_scalar_add(
                s_gamma.rearrange("p g t -> p (g t)"), psG[:], s_bd[:, 0:1]
            )

            # ---- outputs ----
            nc.sync.dma_start(d_hn[:], s_hn[:])
            nc.sync.dma_start(d_cn[:], s_cn[:])
            nc.sync.dma_start(d_lin1[:], s_lin1[:])
            nc.sync.dma_start(d_expl[:], s_expl[:])
            nc.sync.dma_start(d_gamma[:], s_gamma[:])
            nc.sync.dma_start(d_beta[:], s_beta[:])

    nc.compile()
    return nc


def _prep_core_inputs(core, data, Wih0, Whh0, bih0, bhh0, Wih1, Whh1, bih1, bhh1,
                      Wlin, blin, W1, b1, W2, b2, Wd, bd):
    """Host-side shard + retile + bf16 cast for one core."""
    lo, hi = core * GPC, (core + 1) * GPC
    # PyTorch gate order is (i,f,g,o) in 256-blocks = chunks [i,i,f,f,g,g,o,o];
    # the device wants [i,i,f,f,o,o,g,g] (contiguous sigmoid slice).
    GPERM = np.array([0, 1, 2, 3, 6, 7, 4, 5])

    def wtiles(W, cdim):
        # W: [GPC, cdim*128, 256] -> [128(q), GPC, A, cdim, 128(m)]
        arr = W.reshape(GPC, cdim, 128, A, 128)  # [g, c, m, a, q]
        if cdim == C8:
            arr = arr[:, GPERM].copy()
            # tanh(g) is computed as 2*sigmoid(2*g)-1; fold the 2x here
            arr[:, 6:8] *= 2.0
        return np.ascontiguousarray(arr.transpose(4, 0, 3, 1, 2)).astype(BF16)

    def btiles(b, cdim):
        # b: [GPC, cdim*128] -> [128(p), GPC, cdim]
        arr = b.reshape(GPC, cdim, 128)
        if cdim == C8:
            arr = arr[:, GPERM].copy()
            arr[:, 6:8] *= 2.0
        return np.ascontiguousarray(arr.transpose(2, 0, 1)).astype(np.float32)

    d = {}
    x = data[:, lo:hi, :]  # [T, GPC, 256]
    xT = np.ascontiguousarray(
        x.reshape(T, GPC, A, 128).transpose(3, 1, 2, 0)
    ).astype(BF16)
    d["w0"] = np.stack([wtiles(Wih0[lo:hi], C8), wtiles(Whh0[lo:hi], C8)], axis=2)
    d["w1"] = np.stack([wtiles(Wih1[lo:hi], C8), wtiles(Whh1[lo:hi], C8)], axis=2)
    d["wlin"] = wtiles(Wlin[lo:hi], C2)
    # shared heads: [C2*128, 256] -> [128(q), A, C2, 128(m)]
    wsh = {}
    for name, W in (("w1t", W1), ("w2t", W2)):
        arr = W.reshape(C2, 128, A, 128)  # [c2, m, a, q]
        wsh[name] = np.ascontiguousarray(arr.transpose(3, 2, 0, 1)).astype(BF16)
    bbc0 = np.broadcast_to(
        btiles(bih0[lo:hi] + bhh0[lo:hi], C8)[..., None], (128, GPC, C8, T)
    ).astype(BF16)
    bbc1 = np.broadcast_to(
        btiles(bih1[lo:hi] + bhh1[lo:hi], C8)[..., None], (128, GPC, C8, T)
    ).astype(BF16)
    d["sbf"] = np.concatenate(
        [
            xT.reshape(128, -1),
            np.eye(128, dtype=BF16),
            wsh["w1t"].reshape(128, -1),
            wsh["w2t"].reshape(128, -1),
            np.ascontiguousarray(Wd.reshape(A, 128).T).astype(BF16),
            bbc0.reshape(128, -1),
            bbc1.reshape(128, -1),
            np.ascontiguousarray(np.broadcast_to(
                btiles(blin[lo:hi], C2)[..., None], (128, GPC, C2, T)
            )).astype(BF16).reshape(128, -1),
        ],
        axis=1,
    )
    bdcol = np.zeros((128, 1), np.float32)
    bdcol[0, 0] = float(np.asarray(bd).reshape(-1)[0])
    d["sf32"] = np.concatenate(
        [
            btiles(bih0[lo:hi] + bhh0[lo:hi], C8).reshape(128, -1),
            btiles(bih1[lo:hi] + bhh1[lo:hi], C8).reshape(128, -1),
            btiles(blin[lo:hi], C2).reshape(128, -1),
            np.ascontiguousarray(b1.reshape(C2, 128).T).astype(np.float32),
            np.ascontiguousarray(b2.reshape(C2, 128).T).astype(np.float32),
            bdcol,
        ],
        axis=1,
    )
    assert d["sbf"].shape[1] == SBF_N and d["sf32"].shape[1] == SF32_N
    return d


def _get_compiled():
    if "nc" not in _COMPILED:
        _COMPILED["nc"] = _build_nc()
    return _COMPILED["nc"]


def run_device(in_maps, trace=False, tmpdir=None):
    from concourse import bass_utils

    nc = _get_compiled()
    kw = {}
    if trace:
        kw = dict(trace=True, tmpdir=tmpdir)
    res = bass_utils.run_bass_kernel_spmd(
        nc, in_maps, core_ids=list(range(NCORES)), **kw
    )
    return res


def assemble(results):
    """Per-core device outputs -> full reference-shaped outputs."""
    lin1 = np.empty((G, T, H), np.float32)
    expl = np.empty((G, T, H), np.float32)
    beta = np.empty((G, T, H), np.float32)
    gamma = np.empty((G, T, 1), np.float32)
    hN = np.empty((G, 2, H), np.float32)
    cN = np.empty((G, 2, H), np.float32)
    for core in range(NCORES):
        r = results[core]
        lo = core * GPC
        # [128(p), C2, GPC, T] -> [g, t, c2*128+p]
        for name, dst in (("lin1o", lin1), ("explo", expl), ("betao", beta)):
            v = r[name].reshape(128, C2, GPC, T)
            dst[lo : lo + GPC] = v.transpose(2, 3, 1, 0).reshape(GPC, T, H)
        gamma[lo : lo + GPC] = r["gammao"].reshape(GPC, T, 1)
        # [128(p), l, g, a] -> [g, l, a*128+p]
        for name, dst in (("hno", hN), ("cno", cN)):
            v = r[name].reshape(128, 2, GPC, A)
            dst[lo : lo + GPC] = v.transpose(2, 1, 3, 0).reshape(GPC, 2, H)
    delta = expl / expl.sum(axis=0, keepdims=True)
    return gamma, beta, delta, hN, cN


def kernel(**inputs):
    in_maps = [_prep_core_inputs(c, **inputs) for c in range(NCORES)]
    res = run_device(in_maps)
    return assemble(res.results)
